# revision 7
# baseline (speedup 1.0000x reference)
"""Trainium2 Bass kernel for a differentiable-DTW style module.

Math (per batch b):
    dist[i, j] = |score[b, i] - template[j]|              (i, j in [0, 2048))
    path       = softmax(-dist, axis=-1)                  (row-stochastic)
    warped[b]  = path @ feature[b]                        ([2048, 512])
    l2         = 1e-7 * sqrt(sum(warped ** 2))            (global scalar)

Implementation notes:
  - Data-parallel: batch b -> NeuronCore b (B == 8 == n_cores).
  - The softmax row sums are computed EXACTLY on the host with a sorted
    prefix-sum identity:
        sum_j exp(-|s - t_j|) = exp(-s) * sum_{t_j <= s} exp(t_j)
                              + exp(s)  * sum_{t_j >  s} exp(-t_j)
    so the device never needs a partition-axis reduction; it only scales
    the matmul output rows by the reciprocal denominators.
  - The kernel matrix is generated directly in TRANSPOSED layout
    ET[j, i] = exp(-|s_i - t_j|) (template index on partitions), which is
    exactly the lhsT layout the tensor engine wants; feature in natural
    [j, f] layout is the rhs.  out[i, f] = sum_j ET[j, i] * F[j, f].
  - ET is bf16 for the matmul (1 cycle/col vs 2 for fp32); |s - t| is one
    DVE tensor_scalar (subtract then abs_max vs 0), exp is one ScalarE
    activation per [128, 2048] chunk.
  - The scalar l2 is finished on the host from the gathered output.
"""

import numpy as np
import ml_dtypes

B = 8
S = 2048
F = 512
P = 128
NT = S // P  # 16 chunks / output tiles
NCORES = 8
GA = 8  # phase-A output-tile group (== number of PSUM banks)
L2_REG_WEIGHT = 1e-07

_NC_CACHE = {}


def _build_nc():
    import concourse.mybir as mybir
    import concourse.tile as tile
    from concourse import bacc

    f32 = mybir.dt.float32
    bf16 = mybir.dt.bfloat16
    Alu = mybir.AluOpType
    Act = mybir.ActivationFunctionType

    nc = bacc.Bacc(None, target_bir_lowering=False)
    score = nc.dram_tensor("score", [S], f32, kind="ExternalInput")
    negt = nc.dram_tensor("negt", [S], f32, kind="ExternalInput")
    rden = nc.dram_tensor("rden", [S], f32, kind="ExternalInput")
    feat = nc.dram_tensor("feature", [S, F], bf16, kind="ExternalInput")
    warped = nc.dram_tensor("warped", [S, F], f32, kind="ExternalOutput")

    with tile.TileContext(nc) as tc:
        with (
            tc.tile_pool(name="const", bufs=1) as cpool,
            tc.tile_pool(name="feat", bufs=NT) as fpool,
            tc.tile_pool(name="et", bufs=NT) as epool,
            tc.tile_pool(name="dtile", bufs=3) as dpool,
            tc.tile_pool(name="otile", bufs=4) as opool,
            tc.tile_pool(name="ps", bufs=8, space="PSUM") as pspool,
        ):
            # score broadcast to all partitions: s_bcast[p, i] = s_i
            s_bcast = cpool.tile([P, S], f32, tag="sb")
            nc.sync.dma_start(
                out=s_bcast[:], in_=score[:].unsqueeze(0).to_broadcast([P, S])
            )
            # nt_sb[p, c] = -t[c*128 + p]
            nt_sb = cpool.tile([P, NT], f32, tag="tsb")
            nc.sync.dma_start(out=nt_sb[:], in_=negt[:].rearrange("(c p) -> p c", p=P))
            # r_sb[p, m] = 1/den[m*128 + p]
            r_sb = cpool.tile([P, NT], f32, tag="rsb")
            nc.sync.dma_start(out=r_sb[:], in_=rden[:].rearrange("(m p) -> p m", p=P))

            fts = []
            for c in range(NT):
                ft = fpool.tile([P, F], bf16, tag="ft")
                nc.sync.dma_start(out=ft[:], in_=feat[c * P : (c + 1) * P, :])
                fts.append(ft)

            def epilogue(ps, m):
                o = opool.tile([P, F], f32, tag="o")
                nc.vector.tensor_scalar_mul(o[:], ps[:], r_sb[:, m : m + 1])
                nc.sync.dma_start(out=warped[m * P : (m + 1) * P, :], in_=o[:])

            # Phase A: generate ET chunk-by-chunk; as each chunk lands, run
            # the 8 matmuls of output tiles 0..7 (one PSUM bank each).
            psA = [
                pspool.tile([P, F], f32, tag="ps", name=f"psA{m}") for m in range(GA)
            ]
            ets = []
            for c in range(NT):
                d = dpool.tile([P, S], f32, tag="d")
                # d[p, i] = |s_i - t_{c*128+p}|  (ScalarE: Abs(in*1 + bias))
                nc.scalar.activation(
                    out=d[:],
                    in_=s_bcast[:],
                    func=Act.Abs,
                    bias=nt_sb[:, c : c + 1],
                    scale=1.0,
                )
                et = epool.tile([P, S], bf16, tag="et")
                nc.scalar.activation(out=et[:], in_=d[:], func=Act.Exp, scale=-1.0)
                ets.append(et)
                for m in range(GA):
                    nc.tensor.matmul(
                        psA[m][:],
                        et[:, m * P : (m + 1) * P],
                        fts[c][:],
                        start=(c == 0),
                        stop=(c == NT - 1),
                    )
            for m in range(GA):
                epilogue(psA[m], m)

            # Phase B: all ET chunks resident; dense matmuls for tiles 8..15.
            for m in range(GA, NT):
                ps = pspool.tile([P, F], f32, tag="ps", name=f"psB{m}")
                for c in range(NT):
                    nc.tensor.matmul(
                        ps[:],
                        ets[c][:, m * P : (m + 1) * P],
                        fts[c][:],
                        start=(c == 0),
                        stop=(c == NT - 1),
                    )
                epilogue(ps, m)

    nc.compile()
    return nc


def get_nc():
    if "nc" not in _NC_CACHE:
        _NC_CACHE["nc"] = _build_nc()
    return _NC_CACHE["nc"]


def _host_rden(score, template):
    """Exact softmax denominators: rden[b, i] = 1 / sum_j exp(-|s_bi - t_j|)."""
    s = score[:, :, 0].astype(np.float64)  # [B, S]
    t = np.sort(template[0, :, 0].astype(np.float64))  # [S]
    C = np.concatenate([[0.0], np.cumsum(np.exp(t))])  # C[k] = sum_{j<k} e^{t_j}
    D = np.concatenate([[0.0], np.cumsum(np.exp(-t)[::-1])])[::-1]  # sum_{j>=k} e^{-t_j}
    k = np.searchsorted(t, s.ravel(), side="right").reshape(s.shape)
    den = np.exp(-s) * C[k] + np.exp(s) * D[k]
    return (1.0 / den).astype(np.float32)  # [B, S]


def make_in_maps(score, feature, template):
    rden = _host_rden(score, template)
    nt32 = np.ascontiguousarray(-template[0, :, 0], dtype=np.float32)
    in_maps = []
    for b in range(B):
        in_maps.append(
            {
                "score": np.ascontiguousarray(score[b, :, 0], dtype=np.float32),
                "negt": nt32,
                "rden": np.ascontiguousarray(rden[b]),
                "feature": np.asarray(feature[b], dtype=np.float32).astype(
                    ml_dtypes.bfloat16
                ),
            }
        )
    return in_maps


def postprocess(results):
    """results: per-core list of {name: np.ndarray} -> (warped, l2)."""
    warped = np.stack(
        [np.asarray(results[b]["warped"], dtype=np.float32) for b in range(B)]
    )
    l2 = np.float32(
        L2_REG_WEIGHT * np.sqrt(np.sum(warped.astype(np.float64) ** 2))
    )
    return warped, l2


def kernel(score, feature, template):
    from concourse.bass_utils import run_bass_kernel_spmd

    nc = get_nc()
    in_maps = make_in_maps(score, feature, template)
    res = run_bass_kernel_spmd(nc, in_maps, core_ids=list(range(NCORES)))
    return postprocess(res.results)


# revision 12
# speedup vs baseline: 1.2726x; 1.2726x over previous
"""Trainium2 Bass kernel for a differentiable-DTW style module.

Math (per batch b):
    dist[i, j] = |score[b, i] - template[j]|              (i, j in [0, 2048))
    path       = softmax(-dist, axis=-1)                  (row-stochastic)
    warped[b]  = path @ feature[b]                        ([2048, 512])
    l2         = 1e-7 * sqrt(sum(warped ** 2))            (global scalar)

Implementation notes:
  - Data-parallel: batch b -> NeuronCore b (B == 8 == n_cores).
  - The softmax row sums are computed EXACTLY on the host with a sorted
    prefix-sum identity:
        sum_j exp(-|s - t_j|) = exp(-s) * sum_{t_j <= s} exp(t_j)
                              + exp(s)  * sum_{t_j >  s} exp(-t_j)
    so the device never needs a partition-axis reduction; it only scales
    the matmul output rows by the reciprocal denominators.
  - The kernel matrix is generated directly in TRANSPOSED layout
    ET[j, i] = exp(-|s_i - t_j|) (template index on partitions), which is
    exactly the lhsT layout the tensor engine wants; feature in natural
    [j, f] layout is the rhs.  out[i, f] = sum_j ET[j, i] * F[j, f].
  - ET is bf16 for the matmul (1 cycle/col vs 2 for fp32); |s - t| is one
    DVE tensor_scalar (subtract then abs_max vs 0), exp is one ScalarE
    activation per [128, 2048] chunk.
  - The scalar l2 is finished on the host from the gathered output.
"""

import numpy as np
import ml_dtypes

B = 8
S = 2048
F = 512
P = 128
NT = S // P  # 16 chunks / output tiles
NCORES = 8
GA = 8  # phase-A output-tile group (== number of PSUM banks)
L2_REG_WEIGHT = 1e-07

_NC_CACHE = {}


def _build_nc():
    import concourse.mybir as mybir
    import concourse.tile as tile
    from concourse import bacc

    f32 = mybir.dt.float32
    bf16 = mybir.dt.bfloat16
    Alu = mybir.AluOpType
    Act = mybir.ActivationFunctionType

    nc = bacc.Bacc(None, target_bir_lowering=False)
    score = nc.dram_tensor("score", [S], f32, kind="ExternalInput")
    negt = nc.dram_tensor("negt", [S], f32, kind="ExternalInput")
    rden = nc.dram_tensor("rden", [S], f32, kind="ExternalInput")
    # exp(s), exp(-s), exp(-t), exp(t) host-precomputed, bf16 (DVE min-trick)
    pexp = nc.dram_tensor("pexp", [S], bf16, kind="ExternalInput")
    pinv = nc.dram_tensor("pinv", [S], bf16, kind="ExternalInput")
    qexp = nc.dram_tensor("qexp", [S], f32, kind="ExternalInput")
    qinv = nc.dram_tensor("qinv", [S], f32, kind="ExternalInput")
    feat = nc.dram_tensor("feature", [S, F], bf16, kind="ExternalInput")
    warped = nc.dram_tensor("warped", [S, F], f32, kind="ExternalOutput")

    # chunks generated on ScalarE (Abs+Exp); the rest on VectorE via
    # E = min(exp(s)exp(-t), exp(-s)exp(t))  (exactly exp(-|s-t|))
    ACT_CHUNKS = {c for c in range(NT) if c % 2 == 0}

    with tile.TileContext(nc) as tc:
        with (
            tc.tile_pool(name="const", bufs=1) as cpool,
            tc.tile_pool(name="feat", bufs=NT) as fpool,
            tc.tile_pool(name="et", bufs=NT) as epool,
            tc.tile_pool(name="dtile", bufs=3) as dpool,
            tc.tile_pool(name="otile", bufs=4) as opool,
            tc.tile_pool(name="ps", bufs=8, space="PSUM") as pspool,
        ):
            # score broadcast to all partitions: s_bcast[p, i] = s_i
            s_bcast = cpool.tile([P, S], f32, tag="sb")
            nc.sync.dma_start(
                out=s_bcast[:], in_=score[:].unsqueeze(0).to_broadcast([P, S])
            )
            # nt_sb[p, c] = -t[c*128 + p]
            nt_sb = cpool.tile([P, NT], f32, tag="tsb")
            nc.sync.dma_start(out=nt_sb[:], in_=negt[:].rearrange("(c p) -> p c", p=P))
            # r_sb[p, m] = 1/den[m*128 + p]
            r_sb = cpool.tile([P, NT], f32, tag="rsb")
            nc.sync.dma_start(out=r_sb[:], in_=rden[:].rearrange("(m p) -> p m", p=P))
            # broadcast exp(s)/exp(-s) rows (bf16) + per-chunk exp(-t)/exp(t)
            p_bc = cpool.tile([P, S], bf16, tag="pbc")
            nc.sync.dma_start(out=p_bc[:], in_=pexp[:].unsqueeze(0).to_broadcast([P, S]))
            pi_bc = cpool.tile([P, S], bf16, tag="pibc")
            nc.sync.dma_start(
                out=pi_bc[:], in_=pinv[:].unsqueeze(0).to_broadcast([P, S])
            )
            q_sb = cpool.tile([P, NT], f32, tag="qsb")
            nc.sync.dma_start(out=q_sb[:], in_=qexp[:].rearrange("(c p) -> p c", p=P))
            qi_sb = cpool.tile([P, NT], f32, tag="qisb")
            nc.sync.dma_start(out=qi_sb[:], in_=qinv[:].rearrange("(c p) -> p c", p=P))

            fts = []
            for c in range(NT):
                ft = fpool.tile([P, F], bf16, tag="ft")
                nc.sync.dma_start(out=ft[:], in_=feat[c * P : (c + 1) * P, :])
                fts.append(ft)

            # PE warmup: ~10 throwaway matmuls as soon as the first feature
            # tile lands, so HAM un-throttles before the real work.
            wps = pspool.tile([P, F], f32, tag="ps", name="warmup_ps")
            for _ in range(10):
                nc.tensor.matmul(wps[:], fts[0][:, 0:P], fts[0][:], start=True, stop=True)

            def epilogue(ps, m):
                o = opool.tile([P, F], f32, tag="o")
                nc.vector.tensor_scalar_mul(o[:], ps[:], r_sb[:, m : m + 1])
                nc.sync.dma_start(out=warped[m * P : (m + 1) * P, :], in_=o[:])

            # Phase A: generate ET chunk-by-chunk; as each chunk lands, run
            # the 8 matmuls of output tiles 0..7 (one PSUM bank each).
            psA = [
                pspool.tile([P, F], f32, tag="ps", name=f"psA{m}") for m in range(GA)
            ]
            ets = []
            for c in range(NT):
                et = epool.tile([P, S], bf16, tag="et")
                if c in ACT_CHUNKS:
                    d = dpool.tile([P, S], f32, tag="d")
                    # d[p, i] = |s_i - t_{c*128+p}|  (ScalarE: Abs(in*1 + bias))
                    nc.scalar.activation(
                        out=d[:],
                        in_=s_bcast[:],
                        func=Act.Abs,
                        bias=nt_sb[:, c : c + 1],
                        scale=1.0,
                    )
                    nc.scalar.activation(out=et[:], in_=d[:], func=Act.Exp, scale=-1.0)
                else:
                    m1 = dpool.tile([P, S], bf16, tag="m1")
                    m2 = dpool.tile([P, S], bf16, tag="m2")
                    nc.vector.tensor_scalar_mul(m1[:], p_bc[:], q_sb[:, c : c + 1])
                    nc.vector.tensor_scalar_mul(m2[:], pi_bc[:], qi_sb[:, c : c + 1])
                    nc.vector.tensor_tensor(
                        out=et[:], in0=m1[:], in1=m2[:], op=Alu.min
                    )
                ets.append(et)
                for m in range(GA):
                    nc.tensor.matmul(
                        psA[m][:],
                        et[:, m * P : (m + 1) * P],
                        fts[c][:],
                        start=(c == 0),
                        stop=(c == NT - 1),
                    )
            for m in range(GA):
                epilogue(psA[m], m)

            # Phase B: all ET chunks resident; dense matmuls for tiles 8..15.
            for m in range(GA, NT):
                ps = pspool.tile([P, F], f32, tag="ps", name=f"psB{m}")
                for c in range(NT):
                    nc.tensor.matmul(
                        ps[:],
                        ets[c][:, m * P : (m + 1) * P],
                        fts[c][:],
                        start=(c == 0),
                        stop=(c == NT - 1),
                    )
                epilogue(ps, m)

    nc.compile()
    return nc


def get_nc():
    if "nc" not in _NC_CACHE:
        _NC_CACHE["nc"] = _build_nc()
    return _NC_CACHE["nc"]


def _host_rden(score, template):
    """Exact softmax denominators: rden[b, i] = 1 / sum_j exp(-|s_bi - t_j|)."""
    s = score[:, :, 0].astype(np.float64)  # [B, S]
    t = np.sort(template[0, :, 0].astype(np.float64))  # [S]
    C = np.concatenate([[0.0], np.cumsum(np.exp(t))])  # C[k] = sum_{j<k} e^{t_j}
    D = np.concatenate([[0.0], np.cumsum(np.exp(-t)[::-1])])[::-1]  # sum_{j>=k} e^{-t_j}
    k = np.searchsorted(t, s.ravel(), side="right").reshape(s.shape)
    den = np.exp(-s) * C[k] + np.exp(s) * D[k]
    return (1.0 / den).astype(np.float32)  # [B, S]


def make_in_maps(score, feature, template):
    rden = _host_rden(score, template)
    s = np.ascontiguousarray(score[:, :, 0], dtype=np.float32)  # [B, S]
    t = np.ascontiguousarray(template[0, :, 0], dtype=np.float32)  # [S]
    bf = ml_dtypes.bfloat16
    qexp = np.exp(-t.astype(np.float64)).astype(np.float32)
    qinv = np.exp(t.astype(np.float64)).astype(np.float32)
    in_maps = []
    for b in range(B):
        in_maps.append(
            {
                "score": s[b],
                "negt": -t,
                "rden": np.ascontiguousarray(rden[b]),
                "pexp": np.exp(s[b].astype(np.float64)).astype(bf),
                "pinv": np.exp(-s[b].astype(np.float64)).astype(bf),
                "qexp": qexp,
                "qinv": qinv,
                "feature": np.asarray(feature[b], dtype=np.float32).astype(bf),
            }
        )
    return in_maps


def postprocess(results):
    """results: per-core list of {name: np.ndarray} -> (warped, l2)."""
    warped = np.stack(
        [np.asarray(results[b]["warped"], dtype=np.float32) for b in range(B)]
    )
    l2 = np.float32(
        L2_REG_WEIGHT * np.sqrt(np.sum(warped.astype(np.float64) ** 2))
    )
    return warped, l2


def kernel(score, feature, template):
    from concourse.bass_utils import run_bass_kernel_spmd

    nc = get_nc()
    in_maps = make_in_maps(score, feature, template)
    res = run_bass_kernel_spmd(nc, in_maps, core_ids=list(range(NCORES)))
    return postprocess(res.results)
